# revision 7
# baseline (speedup 1.0000x reference)
"""Trainium2 Bass kernel for nn_CrossAttention (single-CLS-query cross attention).

Reference computes, per batch b:
    q = x[b,0,:] @ wq.T                  (single CLS query)
    k = x[b] @ wk.T ; v = x[b] @ wv.T
    out = softmax(q k^T / sqrt(d)) v ; y = out @ wp.T + bp

Because there is a single query token, the huge K/V projections can be
eliminated algebraically:
    scores[b,h,n] = M[b,h,:] . x[b,n,:]   with  M[b,h,:] = (SCALE*q_h) @ Wk_h
    U[b,h,:]     = sum_n attn[b,h,n] x[b,n,:]
    y[b]         = concat_h(U[b,h,:] @ Wv_h.T) @ wp.T + bp
which needs only two streaming passes over x (~2.5 GMAC total) instead of
the 155 GFLOP dense projections.

Distribution: pure data parallel over batch B=32 across 8 cores (4 batches
per core), no collectives.  The scores need x in [C, N] layout (contraction
over C on the PE partition axis) and the weighted sum needs [N, C]; both
operands stream in bfloat16.  Half of each batch's [N, C] data is produced
on-chip by PE-transposing the already-resident [C, N] tiles (bf16 PSUM out,
one copy per 128-row chunk, alternating DVE/ACT), so HBM only carries
1.5 passes over x instead of 2.
"""

import numpy as np

import concourse.bass as bass
import concourse.tile as tile
from concourse import bacc, mybir
from concourse.bass_utils import run_bass_kernel_spmd

# Problem constants (hardcoded per the harness contract).
B, N, C = 32, 4096, 768
H, D = 12, 64
SCALE = D ** -0.5
NCORES = 8
BSH = B // NCORES  # batches per core

F32 = mybir.dt.float32
BF16 = mybir.dt.bfloat16

NCHUNK = C // 128   # 6
NTW = 1024          # phase-A n-window per DMA
NWIN = N // NTW     # 4 windows per batch
CPW = NTW // 128    # 8 n-chunks per window
TPW = 4             # chunks per window transposed on-chip (rest DMA'd)
TCH = NWIN * TPW    # transposed chunks per batch
DPW = CPW - TPW     # chunks per window DMA'd


def build_kernel():
    nc = bacc.Bacc("TRN2", target_bir_lowering=False, debug=False,
                   num_devices=NCORES)

    xT = nc.dram_tensor("xT", [BSH, C, N], BF16, kind="ExternalInput")
    x = nc.dram_tensor("x", [BSH, N, C], BF16, kind="ExternalInput")
    x0T = nc.dram_tensor("x0T", [C, BSH], BF16, kind="ExternalInput")
    wqT = nc.dram_tensor("wqT", [C, C], BF16, kind="ExternalInput")
    wk = nc.dram_tensor("wk", [C, C], BF16, kind="ExternalInput")
    wvT = nc.dram_tensor("wvT", [C, C], BF16, kind="ExternalInput")
    wpT = nc.dram_tensor("wpT", [C, C], BF16, kind="ExternalInput")
    bp = nc.dram_tensor("bp", [1, C], F32, kind="ExternalInput")
    i12 = nc.dram_tensor("i12", [H, H], BF16, kind="ExternalInput")
    i128 = nc.dram_tensor("i128", [128, 128], BF16, kind="ExternalInput")
    y = nc.dram_tensor("y", [BSH, C], F32, kind="ExternalOutput")

    with tile.TileContext(nc) as tc:
        cross_attn_kernel(tc, y.ap(), xT.ap(), x.ap(), x0T.ap(), wqT.ap(),
                          wk.ap(), wvT.ap(), wpT.ap(), bp.ap(), i12.ap(),
                          i128.ap())
    nc.compile()
    return nc


def cross_attn_kernel(tc, y, xT, x, x0T, wqT, wk, wvT, wpT, bp, i12, i128):
    from contextlib import ExitStack
    ctx = ExitStack()
    nc = tc.nc
    with ctx:
        consts = ctx.enter_context(tc.tile_pool(name="consts", bufs=1))
        xa_pool = ctx.enter_context(tc.tile_pool(name="xa", bufs=4))
        xct_pool = ctx.enter_context(tc.tile_pool(name="xct", bufs=2))
        xc_pool = ctx.enter_context(tc.tile_pool(name="xc", bufs=4))
        attn_pool = ctx.enter_context(tc.tile_pool(name="attn", bufs=2))
        small = ctx.enter_context(tc.tile_pool(name="small", bufs=2))
        ps_a = ctx.enter_context(tc.tile_pool(name="ps_a", bufs=2, space="PSUM"))
        ps_x = ctx.enter_context(tc.tile_pool(name="ps_x", bufs=2, space="PSUM"))
        ps_c = ctx.enter_context(tc.tile_pool(name="ps_c", bufs=1, space="PSUM"))
        ps_misc = ctx.enter_context(tc.tile_pool(name="ps_misc", bufs=2, space="PSUM"))

        # ---- constant loads ----
        # All on the scalar HWDGE queue so the sync queue starts streaming
        # x tiles immediately; wvT/wpT are deferred until P4 needs them.
        def load_w(ap_dram, name):
            t = consts.tile([128, NCHUNK, C], BF16, tag=name)
            nc.scalar.dma_start(out=t, in_=ap_dram.rearrange("(a p) o -> p a o", p=128))
            return t

        wqT_sb = load_w(wqT, "wqT_sb")
        wk_sb = load_w(wk, "wk_sb")
        x0T_sb = consts.tile([128, NCHUNK, BSH], BF16)
        nc.scalar.dma_start(out=x0T_sb, in_=x0T.rearrange("(a p) b -> p a b", p=128))
        i12_sb = consts.tile([H, H], BF16)
        nc.scalar.dma_start(out=i12_sb, in_=i12)
        i128_sb = consts.tile([128, 128], BF16)
        nc.scalar.dma_start(out=i128_sb, in_=i128)
        bp_sb = consts.tile([BSH, C], F32)
        nc.scalar.dma_start(
            out=bp_sb,
            in_=bass.AP(tensor=bp.tensor, offset=0, ap=[[0, BSH], [1, C]]),
        )
        qT_sb = consts.tile([128, NCHUNK, BSH], BF16)
        # written by a casting tensor_copy from f32 PSUM, read by phase-A matmul
        mT_sb = consts.tile([128, NCHUNK, BSH, H], BF16)

        # ---- P0a: qT[c_out, b] = wq @ (SCALE * x0^T), contraction over c_in ----
        for co in range(NCHUNK):
            ps_q = ps_misc.tile([128, BSH], F32, tag="misc")
            for ci in range(NCHUNK):
                nc.tensor.matmul(
                    ps_q,
                    lhsT=wqT_sb[:, ci, co * 128:(co + 1) * 128],
                    rhs=x0T_sb[:, ci, :],
                    start=(ci == 0), stop=(ci == NCHUNK - 1),
                )
            nc.vector.tensor_copy(qT_sb[:, co, :], ps_q)

        # ---- P0b: mT[c, b, h] = Wk_h^T @ qT_h  (contraction over d=64) ----
        for ci in range(NCHUNK):
            for h in range(H):
                po = (h % 2) * 64
                ch = h // 2
                ps_m = ps_misc.tile([128, BSH], F32, tag="misc")
                nc.tensor.matmul(
                    ps_m,
                    lhsT=wk_sb[po:po + 64, ch, ci * 128:(ci + 1) * 128],
                    rhs=qT_sb[po:po + 64, ch, :],
                    start=True, stop=True,
                )
                nc.vector.tensor_copy(mT_sb[:, ci, :, h], ps_m)

        ut_all = consts.tile([128, NCHUNK, BSH, H], BF16)  # U^T[c, b, h]

        # ---- per-batch main loop ----
        # Phase A (scores+exp) and phase C (weighted sum) are interleaved per
        # 1024-token window: as soon as a window's attn chunk transposes land,
        # its phase-C matmuls run.  That keeps the post-DMA drain to one
        # window instead of a whole batch.
        def tpw(b, w):
            # last window of the last batch is fully on-chip-transposed so the
            # kernel tail never waits on the xc DMA stream
            return CPW if (b == BSH - 1 and w == NWIN - 1) else TPW

        for b in range(BSH):
            attn = attn_pool.tile([H, N], BF16, tag="attn")
            xcT = xct_pool.tile([128, NWIN * TPW + DPW, C], BF16, tag="xcT")
            attnT = attn_pool.tile([128, N // 128, H], BF16, tag="attnT")
            partials = small.tile([H, N // 512], F32, tag="partials")
            psU0 = ps_c.tile([H, 384], F32, tag="psC0")
            psU1 = ps_c.tile([H, 384], F32, tag="psC1")
            psU = [psU0, psU1]
            tslot = 0
            for w in range(NWIN):
                ntr = tpw(b, w)
                xa = xa_pool.tile([128, NCHUNK, NTW], BF16, tag="xa")
                nc.sync.dma_start(
                    out=xa,
                    in_=xT[b].rearrange("(a p) n -> p a n", p=128)
                         [:, :, w * NTW:(w + 1) * NTW],
                )
                if ntr < CPW:
                    xc = xc_pool.tile([128, CPW - ntr, C], BF16, tag="xc")
                    nc.sync.dma_start(
                        out=xc,
                        in_=x[b, w * NTW + ntr * 128:(w + 1) * NTW, :]
                             .rearrange("(t p) c -> p t c", p=128),
                    )
                # scores + exp; exp is fused into the PSUM->SBUF move (no max
                # subtraction needed: |scores|<8)
                for s in range(NTW // 512):
                    n0 = w * NTW + s * 512
                    ps = ps_a.tile([H, 512], F32, tag="psA")
                    for ci in range(NCHUNK):
                        nc.tensor.matmul(
                            ps,
                            lhsT=mT_sb[:, ci, b, :],
                            rhs=xa[:, ci, s * 512:(s + 1) * 512],
                            start=(ci == 0), stop=(ci == NCHUNK - 1),
                        )
                    nc.scalar.activation(
                        out=attn[:, n0:n0 + 512], in_=ps,
                        func=mybir.ActivationFunctionType.Exp,
                        accum_out=partials[:, n0 // 512:n0 // 512 + 1],
                    )
                # on-chip transpose of the first `ntr` 128-row chunks into the
                # [N, C] layout (bf16 PSUM out, one copy per chunk); these
                # depend only on xa, so they overlap the exp above
                wslot = tslot
                for l in range(ntr):
                    ps_xt = ps_x.tile([128, C], BF16, tag="psx")
                    for ci in range(NCHUNK):
                        nc.tensor.transpose(
                            ps_xt[:, ci * 128:(ci + 1) * 128],
                            in_=xa[:, ci, l * 128:(l + 1) * 128],
                            identity=i128_sb,
                        )
                    if l % 2 == 0:
                        nc.vector.tensor_copy(xcT[:, tslot, :], ps_xt)
                    else:
                        nc.scalar.activation(
                            out=xcT[:, tslot, :], in_=ps_xt,
                            func=mybir.ActivationFunctionType.Copy,
                        )
                    tslot += 1
                # attn chunk transposes for this window (PE via identity); the
                # PSUM->SBUF copy casts to bf16 for the phase-C matmul
                for k in range(CPW):
                    nn = w * CPW + k
                    ps_t = ps_misc.tile([128, H], BF16, tag="misc")
                    nc.tensor.transpose(
                        ps_t, in_=attn[:, nn * 128:(nn + 1) * 128],
                        identity=i12_sb)
                    nc.vector.tensor_copy(attnT[:, nn, :], ps_t)
                # phase C for this window: U[h, c] += sum_n attnT[n, h] x[n, c]
                for k in range(CPW):
                    nn = w * CPW + k
                    src = (xcT[:, wslot + k, :] if k < ntr
                           else xc[:, k - ntr, :])
                    for j in range(2):
                        nc.tensor.matmul(
                            psU[j],
                            lhsT=attnT[:, nn, :],
                            rhs=src[:, j * 384:(j + 1) * 384],
                            start=(w == 0 and k == 0),
                            stop=(w == NWIN - 1 and k == CPW - 1),
                        )

            sums = small.tile([H, 1], F32, tag="sums")
            nc.vector.reduce_sum(sums, partials, axis=mybir.AxisListType.X)
            rsum = small.tile([H, 1], F32, tag="rsum")
            nc.vector.reciprocal(rsum, sums)
            # normalize by softmax sum while moving PSUM -> SBUF
            U_sb = small.tile([H, C], BF16, tag="U")
            for j in range(2):
                nc.vector.tensor_scalar_mul(
                    out=U_sb[:, j * 384:(j + 1) * 384], in0=psU[j], scalar1=rsum,
                )

            # transpose U -> UT[c, h] chunks for the output projections
            for k in range(NCHUNK):
                ps_t = ps_misc.tile([128, H], BF16, tag="misc")
                nc.tensor.transpose(ps_t, in_=U_sb[:, k * 128:(k + 1) * 128],
                                    identity=i12_sb)
                nc.vector.tensor_copy(ut_all[:, k, b, :], ps_t)

        # ---- P4a: ypre[h*64+d, b] = sum_c wvT[c, h*64+d] * UT[c, b, h] ----
        # these ride the sync queue, which is idle after the last xa tile
        wvT_sb = consts.tile([128, NCHUNK, C], BF16, tag="wvT_sb")
        nc.sync.dma_start(out=wvT_sb, in_=wvT.rearrange("(a p) o -> p a o", p=128))
        wpT_sb = consts.tile([128, NCHUNK, C], BF16, tag="wpT_sb")
        nc.sync.dma_start(out=wpT_sb, in_=wpT.rearrange("(a p) o -> p a o", p=128))
        ypT_sb = consts.tile([128, NCHUNK, BSH], BF16)
        for h in range(H):
            ps_yp = ps_misc.tile([64, BSH], F32, tag="misc")
            for k in range(NCHUNK):
                nc.tensor.matmul(
                    ps_yp,
                    lhsT=wvT_sb[:, k, h * 64:(h + 1) * 64],
                    rhs=ut_all[:, k, :, h],
                    start=(k == 0), stop=(k == NCHUNK - 1),
                )
            po = (h % 2) * 64
            nc.vector.tensor_copy(ypT_sb[po:po + 64, h // 2, :], ps_yp)

        # ---- P4b: y[b, c_out] = sum_c ypT[c, b] * wpT[c, c_out] + bp ----
        y_sb = small.tile([BSH, C], F32, tag="y")
        for j in range(2):
            ps_y = ps_misc.tile([BSH, 384], F32, tag="misc")
            for k in range(NCHUNK):
                nc.tensor.matmul(
                    ps_y,
                    lhsT=ypT_sb[:, k, :],
                    rhs=wpT_sb[:, k, j * 384:(j + 1) * 384],
                    start=(k == 0), stop=(k == NCHUNK - 1),
                )
            nc.vector.tensor_add(
                out=y_sb[:, j * 384:(j + 1) * 384],
                in0=ps_y,
                in1=bp_sb[:, j * 384:(j + 1) * 384],
            )
        nc.sync.dma_start(out=y, in_=y_sb)


_CACHE = {}
_BF16 = mybir.dt.np(mybir.dt.bfloat16)


def kernel(x, wq, wk, wv, wp, bp, trace=False):
    x = np.ascontiguousarray(x, dtype=np.float32)
    wq = np.asarray(wq, dtype=np.float32)
    wk = np.asarray(wk, dtype=np.float32)
    wv = np.asarray(wv, dtype=np.float32)
    wp = np.asarray(wp, dtype=np.float32)
    bp = np.asarray(bp, dtype=np.float32)

    if "nc" not in _CACHE:
        _CACHE["nc"] = build_kernel()
    nc = _CACHE["nc"]

    x_sh = x.reshape(NCORES, BSH, N, C)
    wqT = np.ascontiguousarray(wq.T.astype(_BF16))
    wkn = np.ascontiguousarray(wk.astype(_BF16))
    wvT = np.ascontiguousarray(wv.T.astype(_BF16))
    wpT = np.ascontiguousarray(wp.T.astype(_BF16))
    bp2 = np.ascontiguousarray(bp.reshape(1, C))
    i12 = np.eye(H, dtype=np.float32).astype(_BF16)
    i128 = np.eye(128, dtype=np.float32).astype(_BF16)

    in_maps = []
    for k in range(NCORES):
        xs = x_sh[k]
        in_maps.append({
            "xT": np.ascontiguousarray(xs.transpose(0, 2, 1).astype(_BF16)),
            "x": np.ascontiguousarray(xs.astype(_BF16)),
            "x0T": np.ascontiguousarray((xs[:, 0, :] * SCALE).T.astype(_BF16)),
            "wqT": wqT,
            "wk": wkn,
            "wvT": wvT,
            "wpT": wpT,
            "bp": bp2,
            "i12": i12,
            "i128": i128,
        })

    res = run_bass_kernel_spmd(nc, in_maps, core_ids=list(range(NCORES)),
                               trace=trace)
    out = np.concatenate([res.results[k]["y"] for k in range(NCORES)], axis=0)
    out = out.reshape(B, 1, C).astype(np.float32)
    if trace:
        _CACHE["last_exec_time_ns"] = res.exec_time_ns
        _CACHE["last_results"] = res
    return out


# revision 8
# speedup vs baseline: 1.0856x; 1.0856x over previous
"""Trainium2 Bass kernel for nn_CrossAttention (single-CLS-query cross attention).

Reference computes, per batch b:
    q = x[b,0,:] @ wq.T                  (single CLS query)
    k = x[b] @ wk.T ; v = x[b] @ wv.T
    out = softmax(q k^T / sqrt(d)) v ; y = out @ wp.T + bp

Because there is a single query token, the huge K/V projections can be
eliminated algebraically:
    scores[b,h,n] = M[b,h,:] . x[b,n,:]   with  M[b,h,:] = (SCALE*q_h) @ Wk_h
    U[b,h,:]     = sum_n attn[b,h,n] x[b,n,:]
    y[b]         = concat_h(U[b,h,:] @ Wv_h.T) @ wp.T + bp
which needs only two streaming passes over x (~2.5 GMAC total) instead of
the 155 GFLOP dense projections.

Distribution: pure data parallel over batch B=32 across 8 cores (4 batches
per core), no collectives.  The scores need x in [C, N] layout (contraction
over C on the PE partition axis) and the weighted sum needs [N, C]; both
operands stream in bfloat16.  Half of each batch's [N, C] data is produced
on-chip by PE-transposing the already-resident [C, N] tiles (bf16 PSUM out,
one copy per 128-row chunk, alternating DVE/ACT), so HBM only carries
1.5 passes over x instead of 2.
"""

import numpy as np

import concourse.bass as bass
import concourse.tile as tile
from concourse import bacc, mybir
from concourse.bass_utils import run_bass_kernel_spmd

# Problem constants (hardcoded per the harness contract).
B, N, C = 32, 4096, 768
H, D = 12, 64
SCALE = D ** -0.5
NCORES = 8
BSH = B // NCORES  # batches per core

F32 = mybir.dt.float32
BF16 = mybir.dt.bfloat16

NCHUNK = C // 128   # 6
NTW = 1024          # phase-A n-window per DMA
NWIN = N // NTW     # 4 windows per batch
CPW = NTW // 128    # 8 n-chunks per window
TPW = 4             # chunks per window transposed on-chip (rest DMA'd)
TCH = NWIN * TPW    # transposed chunks per batch
DPW = CPW - TPW     # chunks per window DMA'd


def build_kernel():
    nc = bacc.Bacc("TRN2", target_bir_lowering=False, debug=False,
                   num_devices=NCORES)

    xT = nc.dram_tensor("xT", [BSH, C, N], BF16, kind="ExternalInput")
    x = nc.dram_tensor("x", [BSH, N, C], BF16, kind="ExternalInput")
    x0T = nc.dram_tensor("x0T", [C, BSH], BF16, kind="ExternalInput")
    wqT = nc.dram_tensor("wqT", [C, C], BF16, kind="ExternalInput")
    wk = nc.dram_tensor("wk", [C, C], BF16, kind="ExternalInput")
    wvT = nc.dram_tensor("wvT", [C, C], BF16, kind="ExternalInput")
    wpT = nc.dram_tensor("wpT", [C, C], BF16, kind="ExternalInput")
    bp = nc.dram_tensor("bp", [1, C], F32, kind="ExternalInput")
    i12 = nc.dram_tensor("i12", [H, H], BF16, kind="ExternalInput")
    i128 = nc.dram_tensor("i128", [128, 128], BF16, kind="ExternalInput")
    y = nc.dram_tensor("y", [BSH, C], F32, kind="ExternalOutput")

    with tile.TileContext(nc) as tc:
        cross_attn_kernel(tc, y.ap(), xT.ap(), x.ap(), x0T.ap(), wqT.ap(),
                          wk.ap(), wvT.ap(), wpT.ap(), bp.ap(), i12.ap(),
                          i128.ap())
    nc.compile()
    return nc


def cross_attn_kernel(tc, y, xT, x, x0T, wqT, wk, wvT, wpT, bp, i12, i128):
    from contextlib import ExitStack
    ctx = ExitStack()
    nc = tc.nc
    with ctx:
        consts = ctx.enter_context(tc.tile_pool(name="consts", bufs=1))
        xa_pool = ctx.enter_context(tc.tile_pool(name="xa", bufs=4))
        xct_pool = ctx.enter_context(tc.tile_pool(name="xct", bufs=2))
        xc_pool = ctx.enter_context(tc.tile_pool(name="xc", bufs=4))
        attn_pool = ctx.enter_context(tc.tile_pool(name="attn", bufs=2))
        small = ctx.enter_context(tc.tile_pool(name="small", bufs=2))
        ps_a = ctx.enter_context(tc.tile_pool(name="ps_a", bufs=2, space="PSUM"))
        ps_x = ctx.enter_context(tc.tile_pool(name="ps_x", bufs=2, space="PSUM"))
        ps_c = ctx.enter_context(tc.tile_pool(name="ps_c", bufs=1, space="PSUM"))
        ps_misc = ctx.enter_context(tc.tile_pool(name="ps_misc", bufs=2, space="PSUM"))

        # ---- constant loads ----
        # All on the scalar HWDGE queue so the sync queue starts streaming
        # x tiles immediately; wvT/wpT are deferred until P4 needs them.
        def load_w(ap_dram, name):
            t = consts.tile([128, NCHUNK, C], BF16, tag=name)
            nc.scalar.dma_start(out=t, in_=ap_dram.rearrange("(a p) o -> p a o", p=128))
            return t

        wqT_sb = load_w(wqT, "wqT_sb")
        wk_sb = load_w(wk, "wk_sb")
        x0T_sb = consts.tile([128, NCHUNK, BSH], BF16)
        nc.scalar.dma_start(out=x0T_sb, in_=x0T.rearrange("(a p) b -> p a b", p=128))
        i12_sb = consts.tile([H, H], BF16)
        nc.scalar.dma_start(out=i12_sb, in_=i12)
        i128_sb = consts.tile([128, 128], BF16)
        nc.scalar.dma_start(out=i128_sb, in_=i128)
        bp_sb = consts.tile([BSH, C], F32)
        nc.scalar.dma_start(
            out=bp_sb,
            in_=bass.AP(tensor=bp.tensor, offset=0, ap=[[0, BSH], [1, C]]),
        )
        qT_sb = consts.tile([128, NCHUNK, BSH], BF16)
        # written by a casting tensor_copy from f32 PSUM, read by phase-A matmul
        mT_sb = consts.tile([128, NCHUNK, BSH, H], BF16)

        # ---- P0a: qT[c_out, b] = wq @ (SCALE * x0^T), contraction over c_in ----
        ps_q = ps_misc.tile([128, NCHUNK, BSH], F32, tag="misc")
        for co in range(NCHUNK):
            for ci in range(NCHUNK):
                nc.tensor.matmul(
                    ps_q[:, co, :],
                    lhsT=wqT_sb[:, ci, co * 128:(co + 1) * 128],
                    rhs=x0T_sb[:, ci, :],
                    start=(ci == 0), stop=(ci == NCHUNK - 1),
                )
        nc.vector.tensor_copy(qT_sb, ps_q)

        # ---- P0b: mT[c, b, h] = Wk_h^T @ qT_h  (contraction over d=64) ----
        for ci in range(NCHUNK):
            ps_m = ps_misc.tile([128, H, BSH], F32, tag="misc")
            for h in range(H):
                po = (h % 2) * 64
                ch = h // 2
                nc.tensor.matmul(
                    ps_m[:, h, :],
                    lhsT=wk_sb[po:po + 64, ch, ci * 128:(ci + 1) * 128],
                    rhs=qT_sb[po:po + 64, ch, :],
                    start=True, stop=True,
                )
            # copy [128, h, b] -> mT[:, ci, b, h] with a transposing free AP
            nc.vector.tensor_copy(
                mT_sb[:, ci, :, :],
                bass.AP(tensor=ps_m.tensor, offset=ps_m.offset,
                        ap=[ps_m.ap[0], [1, BSH], [BSH, H]]),
            )

        ut_all = consts.tile([128, NCHUNK, BSH, H], BF16)  # U^T[c, b, h]

        # ---- per-batch main loop ----
        # Phase A (scores+exp) and phase C (weighted sum) are interleaved per
        # 1024-token window: as soon as a window's attn chunk transposes land,
        # its phase-C matmuls run.  That keeps the post-DMA drain to one
        # window instead of a whole batch.
        def tpw(b, w):
            # last window of the last batch is fully on-chip-transposed so the
            # kernel tail never waits on the xc DMA stream
            return CPW if (b == BSH - 1 and w == NWIN - 1) else TPW

        for b in range(BSH):
            attn = attn_pool.tile([H, N], BF16, tag="attn")
            xcT = xct_pool.tile([128, NWIN * TPW + DPW, C], BF16, tag="xcT")
            attnT = attn_pool.tile([128, N // 128, H], BF16, tag="attnT")
            partials = small.tile([H, N // 512], F32, tag="partials")
            psU0 = ps_c.tile([H, 384], F32, tag="psC0")
            psU1 = ps_c.tile([H, 384], F32, tag="psC1")
            psU = [psU0, psU1]
            tslot = 0
            for w in range(NWIN):
                ntr = tpw(b, w)
                xa = xa_pool.tile([128, NCHUNK, NTW], BF16, tag="xa")
                nc.sync.dma_start(
                    out=xa,
                    in_=xT[b].rearrange("(a p) n -> p a n", p=128)
                         [:, :, w * NTW:(w + 1) * NTW],
                )
                if ntr < CPW:
                    xc = xc_pool.tile([128, CPW - ntr, C], BF16, tag="xc")
                    nc.sync.dma_start(
                        out=xc,
                        in_=x[b, w * NTW + ntr * 128:(w + 1) * NTW, :]
                             .rearrange("(t p) c -> p t c", p=128),
                    )
                # scores + exp; exp is fused into the PSUM->SBUF move (no max
                # subtraction needed: |scores|<8)
                for s in range(NTW // 512):
                    n0 = w * NTW + s * 512
                    ps = ps_a.tile([H, 512], F32, tag="psA")
                    for ci in range(NCHUNK):
                        nc.tensor.matmul(
                            ps,
                            lhsT=mT_sb[:, ci, b, :],
                            rhs=xa[:, ci, s * 512:(s + 1) * 512],
                            start=(ci == 0), stop=(ci == NCHUNK - 1),
                        )
                    nc.scalar.activation(
                        out=attn[:, n0:n0 + 512], in_=ps,
                        func=mybir.ActivationFunctionType.Exp,
                        accum_out=partials[:, n0 // 512:n0 // 512 + 1],
                    )
                # on-chip transpose of the first `ntr` 128-row chunks into the
                # [N, C] layout (bf16 PSUM out, one copy per chunk); these
                # depend only on xa, so they overlap the exp above
                wslot = tslot
                for l in range(ntr):
                    ps_xt = ps_x.tile([128, C], BF16, tag="psx")
                    for ci in range(NCHUNK):
                        nc.tensor.transpose(
                            ps_xt[:, ci * 128:(ci + 1) * 128],
                            in_=xa[:, ci, l * 128:(l + 1) * 128],
                            identity=i128_sb,
                        )
                    if l % 2 == 0:
                        nc.vector.tensor_copy(xcT[:, tslot, :], ps_xt)
                    else:
                        nc.scalar.activation(
                            out=xcT[:, tslot, :], in_=ps_xt,
                            func=mybir.ActivationFunctionType.Copy,
                        )
                    tslot += 1
                # attn chunk transposes for this window (PE via identity),
                # batched into one PSUM bank tile and moved with one copy
                ps_at = ps_misc.tile([128, CPW, H], BF16, tag="misc")
                for k in range(CPW):
                    nn = w * CPW + k
                    nc.tensor.transpose(
                        ps_at[:, k, :], in_=attn[:, nn * 128:(nn + 1) * 128],
                        identity=i12_sb)
                nc.vector.tensor_copy(attnT[:, w * CPW:(w + 1) * CPW, :], ps_at)
                # phase C for this window: U[h, c] += sum_n attnT[n, h] x[n, c]
                for k in range(CPW):
                    nn = w * CPW + k
                    src = (xcT[:, wslot + k, :] if k < ntr
                           else xc[:, k - ntr, :])
                    for j in range(2):
                        nc.tensor.matmul(
                            psU[j],
                            lhsT=attnT[:, nn, :],
                            rhs=src[:, j * 384:(j + 1) * 384],
                            start=(w == 0 and k == 0),
                            stop=(w == NWIN - 1 and k == CPW - 1),
                        )

            sums = small.tile([H, 1], F32, tag="sums")
            nc.vector.reduce_sum(sums, partials, axis=mybir.AxisListType.X)
            rsum = small.tile([H, 1], F32, tag="rsum")
            nc.vector.reciprocal(rsum, sums)
            # normalize by softmax sum while moving PSUM -> SBUF
            U_sb = small.tile([H, C], BF16, tag="U")
            for j in range(2):
                nc.vector.tensor_scalar_mul(
                    out=U_sb[:, j * 384:(j + 1) * 384], in0=psU[j], scalar1=rsum,
                )

            # transpose U -> UT[c, h] chunks for the output projections
            ps_ut = ps_misc.tile([128, NCHUNK, H], BF16, tag="misc")
            for k in range(NCHUNK):
                nc.tensor.transpose(ps_ut[:, k, :],
                                    in_=U_sb[:, k * 128:(k + 1) * 128],
                                    identity=i12_sb)
            nc.vector.tensor_copy(ut_all[:, :, b, :], ps_ut)

        # ---- P4a: ypre[h*64+d, b] = sum_c wvT[c, h*64+d] * UT[c, b, h] ----
        # these ride the sync queue, which is idle after the last xa tile
        wvT_sb = consts.tile([128, NCHUNK, C], BF16, tag="wvT_sb")
        nc.sync.dma_start(out=wvT_sb, in_=wvT.rearrange("(a p) o -> p a o", p=128))
        wpT_sb = consts.tile([128, NCHUNK, C], BF16, tag="wpT_sb")
        nc.sync.dma_start(out=wpT_sb, in_=wpT.rearrange("(a p) o -> p a o", p=128))
        ypT_sb = consts.tile([128, NCHUNK, BSH], BF16)
        ps_yp = ps_misc.tile([128, NCHUNK, BSH], F32, tag="misc")
        for h in range(H):
            po = (h % 2) * 64
            nc.tensor.matmul(
                ps_yp[po:po + 64, h // 2, :],
                lhsT=wvT_sb[:, 0, h * 64:(h + 1) * 64],
                rhs=ut_all[:, 0, :, h],
                start=True, stop=False,
            )
            for k in range(1, NCHUNK):
                nc.tensor.matmul(
                    ps_yp[po:po + 64, h // 2, :],
                    lhsT=wvT_sb[:, k, h * 64:(h + 1) * 64],
                    rhs=ut_all[:, k, :, h],
                    start=False, stop=(k == NCHUNK - 1),
                )
        nc.vector.tensor_copy(ypT_sb, ps_yp)

        # ---- P4b: y[b, c_out] = sum_c ypT[c, b] * wpT[c, c_out] + bp ----
        y_sb = small.tile([BSH, C], F32, tag="y")
        for j in range(2):
            ps_y = ps_misc.tile([BSH, 384], F32, tag="misc")
            for k in range(NCHUNK):
                nc.tensor.matmul(
                    ps_y,
                    lhsT=ypT_sb[:, k, :],
                    rhs=wpT_sb[:, k, j * 384:(j + 1) * 384],
                    start=(k == 0), stop=(k == NCHUNK - 1),
                )
            nc.vector.tensor_add(
                out=y_sb[:, j * 384:(j + 1) * 384],
                in0=ps_y,
                in1=bp_sb[:, j * 384:(j + 1) * 384],
            )
        nc.sync.dma_start(out=y, in_=y_sb)


_CACHE = {}
_BF16 = mybir.dt.np(mybir.dt.bfloat16)


def kernel(x, wq, wk, wv, wp, bp, trace=False):
    x = np.ascontiguousarray(x, dtype=np.float32)
    wq = np.asarray(wq, dtype=np.float32)
    wk = np.asarray(wk, dtype=np.float32)
    wv = np.asarray(wv, dtype=np.float32)
    wp = np.asarray(wp, dtype=np.float32)
    bp = np.asarray(bp, dtype=np.float32)

    if "nc" not in _CACHE:
        _CACHE["nc"] = build_kernel()
    nc = _CACHE["nc"]

    x_sh = x.reshape(NCORES, BSH, N, C)
    wqT = np.ascontiguousarray(wq.T.astype(_BF16))
    wkn = np.ascontiguousarray(wk.astype(_BF16))
    wvT = np.ascontiguousarray(wv.T.astype(_BF16))
    wpT = np.ascontiguousarray(wp.T.astype(_BF16))
    bp2 = np.ascontiguousarray(bp.reshape(1, C))
    i12 = np.eye(H, dtype=np.float32).astype(_BF16)
    i128 = np.eye(128, dtype=np.float32).astype(_BF16)

    in_maps = []
    for k in range(NCORES):
        xs = x_sh[k]
        in_maps.append({
            "xT": np.ascontiguousarray(xs.transpose(0, 2, 1).astype(_BF16)),
            "x": np.ascontiguousarray(xs.astype(_BF16)),
            "x0T": np.ascontiguousarray((xs[:, 0, :] * SCALE).T.astype(_BF16)),
            "wqT": wqT,
            "wk": wkn,
            "wvT": wvT,
            "wpT": wpT,
            "bp": bp2,
            "i12": i12,
            "i128": i128,
        })

    res = run_bass_kernel_spmd(nc, in_maps, core_ids=list(range(NCORES)),
                               trace=trace)
    out = np.concatenate([res.results[k]["y"] for k in range(NCORES)], axis=0)
    out = out.reshape(B, 1, C).astype(np.float32)
    if trace:
        _CACHE["last_exec_time_ns"] = res.exec_time_ns
        _CACHE["last_results"] = res
    return out
